# revision 15
# baseline (speedup 1.0000x reference)
"""Trainium2 Bass kernel for a Mamba-style SSM single step.

Reference math (fp32):
    delta = softplus(x @ W_delta @ W_dt + b_dt)        [U, D]
    B = x @ W_B ; C = x @ W_C                          [U, N]
    abar = exp(delta[:,:,None] * A[None,:,:])          [U, D, N]
    h_new = abar * h + (delta*x)[:,:,None] * B[:,None,:]
    y = einsum('udn,un->ud', h_new, C) + D_vec * x

Distribution: tensor-parallel over d_inner across 8 cores (1280 each).
Each core redundantly computes the small projections (t = x@W_delta,
B, C) from full x^T / W_delta, then its own d-shard of the state
update. Layout on-chip: partition dim = users (U=128), free = (d, n).
"""

import os
import numpy as np

U, D_IN, RANK, N = 128, 10240, 320, 32
NCORES = 8
DSH = D_IN // NCORES            # 1280 per-core d shard
DT = int(os.environ.get("MAMBA_DT", "128"))   # d-tile size
NT = DSH // DT                  # tiles per core
CH = D_IN // 128                # contraction chunks for projections

_cache = {}


def _build_module():
    import concourse.bass as bass
    import concourse.mybir as mybir
    import concourse.tile as tile
    from concourse import bacc
    from contextlib import ExitStack

    fp32 = mybir.dt.float32
    AF = mybir.ActivationFunctionType
    OP = mybir.AluOpType

    nc = bacc.Bacc(
        "TRN2",
        target_bir_lowering=False,
        debug=False,
        enable_asserts=False,
        num_devices=NCORES,
    )

    # ---- DRAM I/O (per-core) ----
    use_f32r = os.environ.get("MAMBA_F32R", "1") == "1"
    mmdt = mybir.dt.float32r if use_f32r else fp32
    h_d = nc.dram_tensor("h_in", [U, DSH, N], fp32, kind="ExternalInput").ap()
    x_d = nc.dram_tensor("x_sh", [U, DSH], fp32, kind="ExternalInput").ap()
    xt_d = nc.dram_tensor("xT", [D_IN, U], mmdt, kind="ExternalInput").ap()
    wall_d = nc.dram_tensor("wall", [D_IN, RANK + 2 * N], mmdt, kind="ExternalInput").ap()
    wdt_d = nc.dram_tensor("wdt_aug", [RANK + 1, DSH], fp32, kind="ExternalInput").ap()
    a_d = nc.dram_tensor("a_sh", [DSH, N], fp32, kind="ExternalInput").ap()
    dbc_d = nc.dram_tensor("dbc", [U, DSH], fp32, kind="ExternalInput").ap()
    ident_d = nc.dram_tensor("ident", [128, 128], fp32, kind="ExternalInput").ap()
    hnew_d = nc.dram_tensor("h_out", [U, DSH, N], fp32, kind="ExternalOutput").ap()
    y_d = nc.dram_tensor("y_out", [U, DSH], fp32, kind="ExternalOutput").ap()

    with tile.TileContext(nc) as tc, ExitStack() as ctx:
        const = ctx.enter_context(tc.tile_pool(name="const", bufs=1))
        wpool = ctx.enter_context(tc.tile_pool(name="w", bufs=2))
        ppool = ctx.enter_context(tc.tile_pool(name="ps", bufs=2, space="PSUM"))
        hpool = ctx.enter_context(tc.tile_pool(name="h", bufs=2))
        apool = ctx.enter_context(tc.tile_pool(name="abc", bufs=2))
        bpool = ctx.enter_context(tc.tile_pool(name="bx", bufs=2))

        # ---------------- Phase P: projections ----------------
        x_sb = const.tile([U, DSH], fp32, tag="x")
        nc.sync.dma_start(x_sb[:], x_d)
        dbc_sb = const.tile([U, DSH], fp32, tag="dbc")
        nc.sync.dma_start(dbc_sb[:], dbc_d)
        ident = const.tile([128, 128], fp32, tag="ident")
        nc.sync.dma_start(ident[:], ident_d)
        # W_dt_aug rows as three partition-chunks (128/128/65)
        wdt0 = const.tile([128, DSH], fp32, tag="wdt0")
        nc.sync.dma_start(wdt0[:], wdt_d[0:128, :])
        wdt1 = const.tile([128, DSH], fp32, tag="wdt1")
        nc.sync.dma_start(wdt1[:], wdt_d[128:256, :])
        wdt2 = const.tile([65, DSH], fp32, tag="wdt2")
        nc.sync.dma_start(wdt2[:], wdt_d[256:321, :])

        # t|B|C = x @ [W_delta | W_B | W_C]  (accumulate over 80 chunks)
        # float32r runs the PE at 1 cycle/row (vs 4 for fp32) for moving
        # free >= 256.
        tbc_ps = ppool.tile([128, RANK + 2 * N], fp32, tag="tbc")
        KG = 8  # contraction chunks per DMA batch
        W2 = RANK + 2 * N
        xt_g = xt_d.rearrange("(g k p) u -> g p k u", k=KG, p=128)
        wall_g = wall_d.rearrange("(g k p) w -> g p k w", k=KG, p=128)
        for g in range(CH // KG):
            xt_c = wpool.tile([128, KG, U], mmdt, tag="xt")
            nc.sync.dma_start(xt_c[:], xt_g[g])
            w_c = wpool.tile([128, KG, W2], mmdt, tag="wall")
            nc.sync.dma_start(w_c[:], wall_g[g])
            for k in range(KG):
                c = g * KG + k
                nc.tensor.matmul(
                    tbc_ps[:], lhsT=xt_c[:, k, :], rhs=w_c[:, k, :],
                    start=(c == 0), stop=(c == CH - 1),
                )
        t_sb = const.tile([128, RANK], fp32, tag="t")
        nc.scalar.copy(t_sb[:], tbc_ps[:, 0:RANK])
        bc_sb = const.tile([128, 2 * N], fp32, tag="bc")
        nc.scalar.copy(bc_sb[:], tbc_ps[:, RANK:RANK + 2 * N])

        # tT chunks (128/128/64 rows) + ones row for the bias
        tT0 = const.tile([128, U], fp32, tag="tT0")
        tT1 = const.tile([128, U], fp32, tag="tT1")
        tT2 = const.tile([65, U], fp32, tag="tT2")
        for j, (r0, rc, dst) in enumerate([(0, 128, tT0), (128, 128, tT1), (256, 64, tT2)]):
            tt_ps = ppool.tile([rc, 128], fp32, tag="ttps")
            nc.tensor.transpose(tt_ps[:], t_sb[:, r0:r0 + rc], ident[:])
            nc.scalar.copy(dst[0:rc, :], tt_ps[:])
        nc.vector.memset(tT2[64:65, :], 1.0)

        # delta = softplus(tT.T @ W_dt_aug)  per d-tile
        delta_sb = const.tile([U, DSH], fp32, tag="delta")
        for i in range(NT):
            sl = slice(i * DT, (i + 1) * DT)
            d_ps = ppool.tile([U, DT], fp32, tag="dps")
            nc.tensor.matmul(d_ps[:], lhsT=tT0[:], rhs=wdt0[:, sl], start=True, stop=False)
            nc.tensor.matmul(d_ps[:], lhsT=tT1[:], rhs=wdt1[:, sl], start=False, stop=False)
            nc.tensor.matmul(d_ps[:], lhsT=tT2[:], rhs=wdt2[:, sl], start=False, stop=True)
            # softplus(z) = ln(exp(z) + 1) — Exp and Ln share one ACT table
            nc.scalar.activation(d_ps[:], d_ps[:], AF.Exp)
            nc.scalar.activation(delta_sb[:, sl], d_ps[:], AF.Ln, bias=1.0)

        # dx = delta * x ; y init = D * x
        dx_sb = const.tile([U, DSH], fp32, tag="dx")
        nc.vector.tensor_tensor(dx_sb[:], delta_sb[:], x_sb[:], op=OP.mult)
        y_sb = const.tile([U, DSH], fp32, tag="y")
        nc.vector.tensor_tensor(y_sb[:], x_sb[:], dbc_sb[:], op=OP.mult)

        # ---------------- Phase E: state update ----------------
        # A staged once per tile into one partition, then broadcast
        # SBUF->SBUF (keeps the replicated read off HBM).
        B_view = bc_sb[:, 0:N].unsqueeze(1).broadcast_to([U, DT, N])
        C_view = bc_sb[:, N:2 * N].unsqueeze(1).broadcast_to([U, DT, N])
        for i in range(NT):
            sl = slice(i * DT, (i + 1) * DT)
            abc = apool.tile([U, DT, N], fp32, tag="abc")
            a_src = a_d[sl, :].unsqueeze(0).broadcast_to([U, DT, N])
            nc.sync.dma_start(abc[:], a_src)
            # h tile
            ht = hpool.tile([U, DT, N], fp32, tag="ht")
            nc.sync.dma_start(ht[:], h_d[:, sl, :])
            # tmp = delta (x) A   (in place over abc)
            dview = delta_sb[:, sl].unsqueeze(2).broadcast_to([U, DT, N])
            nc.vector.tensor_tensor(abc[:], dview, abc[:], op=OP.mult)
            # abar = exp(tmp)  (in place)
            nc.scalar.activation(abc[:], abc[:], AF.Exp)
            # ah = abar * h    (in place over ht)
            nc.vector.tensor_tensor(ht[:], abc[:], ht[:], op=OP.mult)
            # bx = dx (x) B  (into bxt). Runs on GpSimd: fp32 tensor_tensor
            # on DVE is 1x-mode (dedicated ports), so GpSimd never contends
            # and this overlaps with the DVE passes.
            dxview = dx_sb[:, sl].unsqueeze(2).broadcast_to([U, DT, N])
            bxt = bpool.tile([U, DT, N], fp32, tag="bxt")
            nc.gpsimd.tensor_tensor(bxt[:], dxview, B_view, op=OP.mult)
            # h_new = ah + bx  (also on GpSimd)
            nc.gpsimd.tensor_tensor(ht[:], bxt[:], ht[:], op=OP.add)
            # q = h_new * C  (into abc)
            nc.vector.tensor_tensor(abc[:], ht[:], C_view, op=OP.mult)
            # y[:, sl] += sum_n q
            yp = wpool.tile([U, DT], fp32, tag="yp")
            nc.vector.tensor_reduce(yp[:], abc[:], axis=mybir.AxisListType.X, op=OP.add)
            nc.vector.tensor_tensor(y_sb[:, sl], yp[:], y_sb[:, sl], op=OP.add)
            nc.sync.dma_start(hnew_d[:, sl, :], ht[:])
        nc.sync.dma_start(y_d, y_sb[:])

    nc.compile()
    return nc


def _get_module():
    if "nc" not in _cache:
        _cache["nc"] = _build_module()
    return _cache["nc"]


def _make_in_maps(x, h, W_delta, W_dt, b_dt, W_B, W_C, A, D):
    x = np.ascontiguousarray(np.asarray(x, np.float32))
    h = np.asarray(h, np.float32)
    wall = np.ascontiguousarray(
        np.concatenate(
            [np.asarray(W_delta, np.float32), np.asarray(W_B, np.float32),
             np.asarray(W_C, np.float32)], axis=1)
    )
    xt = np.ascontiguousarray(x.T)
    wdt_aug = np.ascontiguousarray(
        np.concatenate(
            [np.asarray(W_dt, np.float32),
             np.asarray(b_dt, np.float32)[None, :]], axis=0)
    )
    A = np.asarray(A, np.float32)
    D = np.asarray(D, np.float32)
    ident = np.eye(128, dtype=np.float32)
    in_maps = []
    for k in range(NCORES):
        sl = slice(k * DSH, (k + 1) * DSH)
        in_maps.append({
            "h_in": np.ascontiguousarray(h[:, sl, :]),
            "x_sh": np.ascontiguousarray(x[:, sl]),
            "xT": xt,
            "wall": wall,
            "wdt_aug": np.ascontiguousarray(wdt_aug[:, sl]),
            "a_sh": np.ascontiguousarray(A[sl, :]),
            "dbc": np.ascontiguousarray(
                np.broadcast_to(D[sl][None, :], (U, DSH))),
            "ident": ident,
        })
    return in_maps


def _run(in_maps, trace=False):
    from concourse import bass_utils
    nc = _get_module()
    res = bass_utils.run_bass_kernel_spmd(
        nc, in_maps, core_ids=list(range(NCORES)), trace=trace,
    )
    return res


def _gather(results):
    y = np.concatenate([results[k]["y_out"] for k in range(NCORES)], axis=1)
    h_new = np.concatenate([results[k]["h_out"] for k in range(NCORES)], axis=1)
    return y.astype(np.float32), h_new.astype(np.float32)


def kernel(x, h, W_delta, W_dt, b_dt, W_B, W_C, A, D):
    in_maps = _make_in_maps(x, h, W_delta, W_dt, b_dt, W_B, W_C, A, D)
    res = _run(in_maps, trace=False)
    return _gather(res.results)


def _install_ntff_hook():
    """Shim antenv.axon_hooks (absent in this image) and register the
    ctypes NTFF profile hook so trace=True yields exec_time_ns."""
    import sys
    import types
    if "antenv.axon_hooks" not in sys.modules:
        import antenv
        mod = types.ModuleType("antenv.axon_hooks")
        mod._hook = None

        def set_axon_ntff_profile_hook(h):
            mod._hook = h

        def get_axon_ntff_profile_hook():
            return mod._hook

        mod.set_axon_ntff_profile_hook = set_axon_ntff_profile_hook
        mod.get_axon_ntff_profile_hook = get_axon_ntff_profile_hook
        sys.modules["antenv.axon_hooks"] = mod
        antenv.axon_hooks = mod
    import antenv.axon_hooks as ah
    if ah._hook is None:
        from trn_agent_boot.trn_boot import _ntff_profile_via_ctypes
        hook = _ntff_profile_via_ctypes("/opt/axon/libaxon_pjrt.so")
        if hook is not None:
            ah.set_axon_ntff_profile_hook(hook)
    # avoid network-dependent artifact upload in the trace path
    from concourse import bass_utils
    bass_utils.upload_artifacts = lambda tmpdir: f"local:{tmpdir}"


def kernel_traced(x, h, W_delta, W_dt, b_dt, W_B, W_C, A, D):
    """Like kernel() but with NTFF tracing; returns ((y, h_new), results)."""
    _install_ntff_hook()
    in_maps = _make_in_maps(x, h, W_delta, W_dt, b_dt, W_B, W_C, A, D)
    res = _run(in_maps, trace=True)
    return _gather(res.results), res


# revision 16
# speedup vs baseline: 1.0296x; 1.0296x over previous
"""Trainium2 Bass kernel for a Mamba-style SSM single step.

Reference math (fp32):
    delta = softplus(x @ W_delta @ W_dt + b_dt)        [U, D]
    B = x @ W_B ; C = x @ W_C                          [U, N]
    abar = exp(delta[:,:,None] * A[None,:,:])          [U, D, N]
    h_new = abar * h + (delta*x)[:,:,None] * B[:,None,:]
    y = einsum('udn,un->ud', h_new, C) + D_vec * x

Distribution: tensor-parallel over d_inner across 8 cores (1280 each).
Each core redundantly computes the small projections (t = x@W_delta,
B, C) from full x^T / W_delta, then its own d-shard of the state
update. Layout on-chip: partition dim = users (U=128), free = (d, n).
"""

import os
import numpy as np

U, D_IN, RANK, N = 128, 10240, 320, 32
NCORES = 8
DSH = D_IN // NCORES            # 1280 per-core d shard
DT = int(os.environ.get("MAMBA_DT", "128"))   # d-tile size
NT = DSH // DT                  # tiles per core
CH = D_IN // 128                # contraction chunks for projections

_cache = {}


def _build_module():
    import concourse.bass as bass
    import concourse.mybir as mybir
    import concourse.tile as tile
    from concourse import bacc
    from contextlib import ExitStack

    fp32 = mybir.dt.float32
    AF = mybir.ActivationFunctionType
    OP = mybir.AluOpType

    nc = bacc.Bacc(
        "TRN2",
        target_bir_lowering=False,
        debug=False,
        enable_asserts=False,
        num_devices=NCORES,
    )

    # ---- DRAM I/O (per-core) ----
    use_f32r = os.environ.get("MAMBA_F32R", "1") == "1"
    mmdt = mybir.dt.float32r if use_f32r else fp32
    h_d = nc.dram_tensor("h_in", [U, DSH, N], fp32, kind="ExternalInput").ap()
    x_d = nc.dram_tensor("x_sh", [U, DSH], fp32, kind="ExternalInput").ap()
    xt_d = nc.dram_tensor("xT", [D_IN, U], mmdt, kind="ExternalInput").ap()
    wall_d = nc.dram_tensor("wall", [D_IN, RANK + 2 * N], mmdt, kind="ExternalInput").ap()
    wdt_d = nc.dram_tensor("wdt_aug", [RANK + 1, DSH], fp32, kind="ExternalInput").ap()
    a_d = nc.dram_tensor("a_sh", [DSH, N], fp32, kind="ExternalInput").ap()
    dbc_d = nc.dram_tensor("dbc", [U, DSH], fp32, kind="ExternalInput").ap()
    ident_d = nc.dram_tensor("ident", [128, 128], fp32, kind="ExternalInput").ap()
    hnew_d = nc.dram_tensor("h_out", [U, DSH, N], fp32, kind="ExternalOutput").ap()
    y_d = nc.dram_tensor("y_out", [U, DSH], fp32, kind="ExternalOutput").ap()

    with tile.TileContext(nc) as tc, ExitStack() as ctx:
        const = ctx.enter_context(tc.tile_pool(name="const", bufs=1))
        wpool = ctx.enter_context(tc.tile_pool(name="w", bufs=2))
        ppool = ctx.enter_context(tc.tile_pool(name="ps", bufs=2, space="PSUM"))
        hpool = ctx.enter_context(tc.tile_pool(name="h", bufs=2))
        apool = ctx.enter_context(tc.tile_pool(name="abc", bufs=2))
        bpool = ctx.enter_context(tc.tile_pool(name="bx", bufs=2))

        # ---------------- Phase P: projections ----------------
        x_sb = const.tile([U, DSH], fp32, tag="x")
        nc.sync.dma_start(x_sb[:], x_d)
        dbc_sb = const.tile([U, DSH], fp32, tag="dbc")
        nc.sync.dma_start(dbc_sb[:], dbc_d)
        ident = const.tile([128, 128], fp32, tag="ident")
        nc.sync.dma_start(ident[:], ident_d)
        # W_dt_aug rows as three partition-chunks (128/128/65)
        wdt0 = const.tile([128, DSH], fp32, tag="wdt0")
        nc.sync.dma_start(wdt0[:], wdt_d[0:128, :])
        wdt1 = const.tile([128, DSH], fp32, tag="wdt1")
        nc.sync.dma_start(wdt1[:], wdt_d[128:256, :])
        wdt2 = const.tile([65, DSH], fp32, tag="wdt2")
        nc.sync.dma_start(wdt2[:], wdt_d[256:321, :])

        # t|B|C = x @ [W_delta | W_B | W_C]  (accumulate over 80 chunks)
        # float32r runs the PE at 1 cycle/row (vs 4 for fp32) for moving
        # free >= 256.
        tbc_ps = ppool.tile([128, RANK + 2 * N], fp32, tag="tbc")
        KG = 8  # contraction chunks per DMA batch
        W2 = RANK + 2 * N
        xt_g = xt_d.rearrange("(g k p) u -> g p k u", k=KG, p=128)
        wall_g = wall_d.rearrange("(g k p) w -> g p k w", k=KG, p=128)
        for g in range(CH // KG):
            xt_c = wpool.tile([128, KG, U], mmdt, tag="xt")
            nc.sync.dma_start(xt_c[:], xt_g[g])
            w_c = wpool.tile([128, KG, W2], mmdt, tag="wall")
            nc.sync.dma_start(w_c[:], wall_g[g])
            for k in range(KG):
                c = g * KG + k
                nc.tensor.matmul(
                    tbc_ps[:], lhsT=xt_c[:, k, :], rhs=w_c[:, k, :],
                    start=(c == 0), stop=(c == CH - 1),
                )
        t_sb = const.tile([128, RANK], fp32, tag="t")
        nc.scalar.copy(t_sb[:], tbc_ps[:, 0:RANK])
        bc_sb = const.tile([128, 2 * N], fp32, tag="bc")
        nc.scalar.copy(bc_sb[:], tbc_ps[:, RANK:RANK + 2 * N])

        # tT chunks (128/128/64 rows) + ones row for the bias
        tT0 = const.tile([128, U], fp32, tag="tT0")
        tT1 = const.tile([128, U], fp32, tag="tT1")
        tT2 = const.tile([65, U], fp32, tag="tT2")
        for j, (r0, rc, dst) in enumerate([(0, 128, tT0), (128, 128, tT1), (256, 64, tT2)]):
            tt_ps = ppool.tile([rc, 128], fp32, tag="ttps")
            nc.tensor.transpose(tt_ps[:], t_sb[:, r0:r0 + rc], ident[:])
            nc.scalar.copy(dst[0:rc, :], tt_ps[:])
        nc.vector.memset(tT2[64:65, :], 1.0)

        # delta = softplus(tT.T @ W_dt_aug)  per d-tile
        delta_sb = const.tile([U, DSH], fp32, tag="delta")
        for i in range(NT):
            sl = slice(i * DT, (i + 1) * DT)
            d_ps = ppool.tile([U, DT], fp32, tag="dps")
            nc.tensor.matmul(d_ps[:], lhsT=tT0[:], rhs=wdt0[:, sl], start=True, stop=False)
            nc.tensor.matmul(d_ps[:], lhsT=tT1[:], rhs=wdt1[:, sl], start=False, stop=False)
            nc.tensor.matmul(d_ps[:], lhsT=tT2[:], rhs=wdt2[:, sl], start=False, stop=True)
            # softplus(z) = ln(exp(z) + 1) — Exp and Ln share one ACT table
            nc.scalar.activation(d_ps[:], d_ps[:], AF.Exp)
            nc.scalar.activation(delta_sb[:, sl], d_ps[:], AF.Ln, bias=1.0)

        # dx = delta * x ; y init = D * x
        dx_sb = const.tile([U, DSH], fp32, tag="dx")
        nc.vector.tensor_tensor(dx_sb[:], delta_sb[:], x_sb[:], op=OP.mult)
        y_sb = const.tile([U, DSH], fp32, tag="y")
        nc.vector.tensor_tensor(y_sb[:], x_sb[:], dbc_sb[:], op=OP.mult)

        # ---------------- Phase E: state update ----------------
        # A staged once per tile into one partition, then broadcast
        # SBUF->SBUF (keeps the replicated read off HBM).
        B_view = bc_sb[:, 0:N].unsqueeze(1).broadcast_to([U, DT, N])
        C_view = bc_sb[:, N:2 * N].unsqueeze(1).broadcast_to([U, DT, N])
        for i in range(NT):
            sl = slice(i * DT, (i + 1) * DT)
            abc = apool.tile([U, DT, N], fp32, tag="abc")
            a_src = a_d[sl, :].unsqueeze(0).broadcast_to([U, DT, N])
            nc.sync.dma_start(abc[:], a_src)
            # h tile
            ht = hpool.tile([U, DT, N], fp32, tag="ht")
            nc.sync.dma_start(ht[:], h_d[:, sl, :])
            # tmp = delta (x) A   (in place over abc)
            dview = delta_sb[:, sl].unsqueeze(2).broadcast_to([U, DT, N])
            nc.vector.tensor_tensor(abc[:], dview, abc[:], op=OP.mult)
            # abar = exp(tmp)  (in place)
            nc.scalar.activation(abc[:], abc[:], AF.Exp)
            # ah = abar * h    (in place over ht)
            nc.vector.tensor_tensor(ht[:], abc[:], ht[:], op=OP.mult)
            # bx = dx (x) B  (into bxt). Runs on GpSimd: fp32 tensor_tensor
            # on DVE is 1x-mode (dedicated ports), so GpSimd never contends
            # and this overlaps with the DVE passes.
            dxview = dx_sb[:, sl].unsqueeze(2).broadcast_to([U, DT, N])
            bxt = bpool.tile([U, DT, N], fp32, tag="bxt")
            nc.gpsimd.tensor_tensor(bxt[:], dxview, B_view, op=OP.mult)
            # h_new = ah + bx
            nc.vector.tensor_tensor(ht[:], bxt[:], ht[:], op=OP.add)
            # q = h_new * C  (into abc)
            nc.vector.tensor_tensor(abc[:], ht[:], C_view, op=OP.mult)
            # y[:, sl] += sum_n q
            yp = wpool.tile([U, DT], fp32, tag="yp")
            nc.vector.tensor_reduce(yp[:], abc[:], axis=mybir.AxisListType.X, op=OP.add)
            nc.vector.tensor_tensor(y_sb[:, sl], yp[:], y_sb[:, sl], op=OP.add)
            nc.sync.dma_start(hnew_d[:, sl, :], ht[:])
        nc.sync.dma_start(y_d, y_sb[:])

    nc.compile()
    return nc


def _get_module():
    if "nc" not in _cache:
        _cache["nc"] = _build_module()
    return _cache["nc"]


def _make_in_maps(x, h, W_delta, W_dt, b_dt, W_B, W_C, A, D):
    x = np.ascontiguousarray(np.asarray(x, np.float32))
    h = np.asarray(h, np.float32)
    wall = np.ascontiguousarray(
        np.concatenate(
            [np.asarray(W_delta, np.float32), np.asarray(W_B, np.float32),
             np.asarray(W_C, np.float32)], axis=1)
    )
    xt = np.ascontiguousarray(x.T)
    wdt_aug = np.ascontiguousarray(
        np.concatenate(
            [np.asarray(W_dt, np.float32),
             np.asarray(b_dt, np.float32)[None, :]], axis=0)
    )
    A = np.asarray(A, np.float32)
    D = np.asarray(D, np.float32)
    ident = np.eye(128, dtype=np.float32)
    in_maps = []
    for k in range(NCORES):
        sl = slice(k * DSH, (k + 1) * DSH)
        in_maps.append({
            "h_in": np.ascontiguousarray(h[:, sl, :]),
            "x_sh": np.ascontiguousarray(x[:, sl]),
            "xT": xt,
            "wall": wall,
            "wdt_aug": np.ascontiguousarray(wdt_aug[:, sl]),
            "a_sh": np.ascontiguousarray(A[sl, :]),
            "dbc": np.ascontiguousarray(
                np.broadcast_to(D[sl][None, :], (U, DSH))),
            "ident": ident,
        })
    return in_maps


def _run(in_maps, trace=False):
    from concourse import bass_utils
    nc = _get_module()
    res = bass_utils.run_bass_kernel_spmd(
        nc, in_maps, core_ids=list(range(NCORES)), trace=trace,
    )
    return res


def _gather(results):
    y = np.concatenate([results[k]["y_out"] for k in range(NCORES)], axis=1)
    h_new = np.concatenate([results[k]["h_out"] for k in range(NCORES)], axis=1)
    return y.astype(np.float32), h_new.astype(np.float32)


def kernel(x, h, W_delta, W_dt, b_dt, W_B, W_C, A, D):
    in_maps = _make_in_maps(x, h, W_delta, W_dt, b_dt, W_B, W_C, A, D)
    res = _run(in_maps, trace=False)
    return _gather(res.results)


def _install_ntff_hook():
    """Shim antenv.axon_hooks (absent in this image) and register the
    ctypes NTFF profile hook so trace=True yields exec_time_ns."""
    import sys
    import types
    if "antenv.axon_hooks" not in sys.modules:
        import antenv
        mod = types.ModuleType("antenv.axon_hooks")
        mod._hook = None

        def set_axon_ntff_profile_hook(h):
            mod._hook = h

        def get_axon_ntff_profile_hook():
            return mod._hook

        mod.set_axon_ntff_profile_hook = set_axon_ntff_profile_hook
        mod.get_axon_ntff_profile_hook = get_axon_ntff_profile_hook
        sys.modules["antenv.axon_hooks"] = mod
        antenv.axon_hooks = mod
    import antenv.axon_hooks as ah
    if ah._hook is None:
        from trn_agent_boot.trn_boot import _ntff_profile_via_ctypes
        hook = _ntff_profile_via_ctypes("/opt/axon/libaxon_pjrt.so")
        if hook is not None:
            ah.set_axon_ntff_profile_hook(hook)
    # avoid network-dependent artifact upload in the trace path
    from concourse import bass_utils
    bass_utils.upload_artifacts = lambda tmpdir: f"local:{tmpdir}"


def kernel_traced(x, h, W_delta, W_dt, b_dt, W_B, W_C, A, D):
    """Like kernel() but with NTFF tracing; returns ((y, h_new), results)."""
    _install_ntff_hook()
    in_maps = _make_in_maps(x, h, W_delta, W_dt, b_dt, W_B, W_C, A, D)
    res = _run(in_maps, trace=True)
    return _gather(res.results), res


# revision 19
# speedup vs baseline: 1.2726x; 1.2360x over previous
"""Trainium2 Bass kernel for a Mamba-style SSM single step.

Reference math (fp32):
    delta = softplus(x @ W_delta @ W_dt + b_dt)        [U, D]
    B = x @ W_B ; C = x @ W_C                          [U, N]
    abar = exp(delta[:,:,None] * A[None,:,:])          [U, D, N]
    h_new = abar * h + (delta*x)[:,:,None] * B[:,None,:]
    y = einsum('udn,un->ud', h_new, C) + D_vec * x

Two SPMD launches over 8 cores:
  1. proj: [t|B|C] = x @ [W_delta|W_B|W_C], column-sharded (48/core).
     Host gathers the 128x384 result and transposes t.
  2. main: tensor-parallel over d_inner (1280/core): delta = softplus
     (t @ W_dt + b), then the elementwise state update in a layout with
     partition dim = users (U=128), free = (d, n).
"""

import os
import numpy as np

U, D_IN, RANK, N = 128, 10240, 320, 32
NCORES = 8
DSH = D_IN // NCORES            # 1280 per-core d shard
DT = int(os.environ.get("MAMBA_DT", "128"))   # d-tile size
NT = DSH // DT                  # tiles per core
CH = D_IN // 128                # contraction chunks for projections
W2 = RANK + 2 * N               # 384
WSL = W2 // NCORES              # 48 proj columns per core

_cache = {}


def _build_proj():
    import concourse.mybir as mybir
    import concourse.tile as tile
    from concourse import bacc
    from contextlib import ExitStack

    fp32 = mybir.dt.float32
    nc = bacc.Bacc("TRN2", target_bir_lowering=False, debug=False,
                   enable_asserts=False, num_devices=NCORES)
    xt_d = nc.dram_tensor("xT", [D_IN, U], fp32, kind="ExternalInput").ap()
    w_d = nc.dram_tensor("wsl", [D_IN, WSL], fp32, kind="ExternalInput").ap()
    o_d = nc.dram_tensor("tbc_out", [128, WSL], fp32, kind="ExternalOutput").ap()

    KG = 8
    with tile.TileContext(nc) as tc, ExitStack() as ctx:
        wpool = ctx.enter_context(tc.tile_pool(name="w", bufs=3))
        ppool = ctx.enter_context(tc.tile_pool(name="ps", bufs=1, space="PSUM"))
        spool = ctx.enter_context(tc.tile_pool(name="s", bufs=1))
        ps = ppool.tile([128, WSL], fp32, tag="acc")
        xt_g = xt_d.rearrange("(g k p) u -> g p k u", k=KG, p=128)
        w_g = w_d.rearrange("(g k p) w -> g p k w", k=KG, p=128)
        for g in range(CH // KG):
            xt_c = wpool.tile([128, KG, U], fp32, tag="xt")
            nc.sync.dma_start(xt_c[:], xt_g[g])
            w_c = wpool.tile([128, KG, WSL], fp32, tag="w")
            nc.sync.dma_start(w_c[:], w_g[g])
            for k in range(KG):
                c = g * KG + k
                nc.tensor.matmul(ps[:], lhsT=xt_c[:, k, :], rhs=w_c[:, k, :],
                                 start=(c == 0), stop=(c == CH - 1))
        out_sb = spool.tile([128, WSL], fp32, tag="out")
        nc.scalar.copy(out_sb[:], ps[:])
        nc.sync.dma_start(o_d, out_sb[:])
    nc.compile()
    return nc


def _build_main():
    import concourse.mybir as mybir
    import concourse.tile as tile
    from concourse import bacc
    from contextlib import ExitStack

    fp32 = mybir.dt.float32
    AF = mybir.ActivationFunctionType
    OP = mybir.AluOpType

    nc = bacc.Bacc("TRN2", target_bir_lowering=False, debug=False,
                   enable_asserts=False, num_devices=NCORES)

    h_d = nc.dram_tensor("h_in", [U, DSH, N], fp32, kind="ExternalInput").ap()
    x_d = nc.dram_tensor("x_sh", [U, DSH], fp32, kind="ExternalInput").ap()
    tta_d = nc.dram_tensor("tTa", [RANK + 1, U], fp32, kind="ExternalInput").ap()
    bc_d = nc.dram_tensor("bc_in", [128, 2 * N], fp32, kind="ExternalInput").ap()
    wdt_d = nc.dram_tensor("wdt_aug", [RANK + 1, DSH], fp32, kind="ExternalInput").ap()
    a_d = nc.dram_tensor("a_sh", [DSH, N], fp32, kind="ExternalInput").ap()
    dbc_d = nc.dram_tensor("dbc", [U, DSH], fp32, kind="ExternalInput").ap()
    hnew_d = nc.dram_tensor("h_out", [U, DSH, N], fp32, kind="ExternalOutput").ap()
    y_d = nc.dram_tensor("y_out", [U, DSH], fp32, kind="ExternalOutput").ap()

    with tile.TileContext(nc) as tc, ExitStack() as ctx:
        const = ctx.enter_context(tc.tile_pool(name="const", bufs=1))
        ppool = ctx.enter_context(tc.tile_pool(name="ps", bufs=2, space="PSUM"))
        hpool = ctx.enter_context(tc.tile_pool(name="h", bufs=2))
        apool = ctx.enter_context(tc.tile_pool(name="abc", bufs=2))
        bpool = ctx.enter_context(tc.tile_pool(name="bx", bufs=2))
        ypool = ctx.enter_context(tc.tile_pool(name="yp", bufs=2))

        # ---------------- Phase P: delta projection ----------------
        x_sb = const.tile([U, DSH], fp32, tag="x")
        nc.sync.dma_start(x_sb[:], x_d)
        dbc_sb = const.tile([U, DSH], fp32, tag="dbc")
        nc.sync.dma_start(dbc_sb[:], dbc_d)
        wdt0 = const.tile([128, DSH], fp32, tag="wdt0")
        nc.sync.dma_start(wdt0[:], wdt_d[0:128, :])
        wdt1 = const.tile([128, DSH], fp32, tag="wdt1")
        nc.sync.dma_start(wdt1[:], wdt_d[128:256, :])
        wdt2 = const.tile([65, DSH], fp32, tag="wdt2")
        nc.sync.dma_start(wdt2[:], wdt_d[256:321, :])
        tT0 = const.tile([128, U], fp32, tag="tT0")
        nc.sync.dma_start(tT0[:], tta_d[0:128, :])
        tT1 = const.tile([128, U], fp32, tag="tT1")
        nc.sync.dma_start(tT1[:], tta_d[128:256, :])
        tT2 = const.tile([65, U], fp32, tag="tT2")
        nc.sync.dma_start(tT2[:], tta_d[256:321, :])
        bc_sb = const.tile([128, 2 * N], fp32, tag="bc")
        nc.sync.dma_start(bc_sb[:], bc_d)

        # delta = softplus(tT.T @ W_dt_aug): per-tile Exp, one Ln pass
        delta_sb = const.tile([U, DSH], fp32, tag="delta")
        for i in range(NT):
            sl = slice(i * DT, (i + 1) * DT)
            d_ps = ppool.tile([U, DT], fp32, tag="dps")
            nc.tensor.matmul(d_ps[:], lhsT=tT0[:], rhs=wdt0[:, sl], start=True, stop=False)
            nc.tensor.matmul(d_ps[:], lhsT=tT1[:], rhs=wdt1[:, sl], start=False, stop=False)
            nc.tensor.matmul(d_ps[:], lhsT=tT2[:], rhs=wdt2[:, sl], start=False, stop=True)
            nc.scalar.activation(delta_sb[:, sl], d_ps[:], AF.Exp)
        nc.scalar.activation(delta_sb[:], delta_sb[:], AF.Ln, bias=1.0)

        # dx = delta * x ; y init = D * x
        dx_sb = const.tile([U, DSH], fp32, tag="dx")
        nc.vector.tensor_tensor(dx_sb[:], delta_sb[:], x_sb[:], op=OP.mult)
        y_sb = const.tile([U, DSH], fp32, tag="y")
        nc.vector.tensor_tensor(y_sb[:], x_sb[:], dbc_sb[:], op=OP.mult)

        # ---------------- Phase E: state update ----------------
        B_view = bc_sb[:, 0:N].unsqueeze(1).broadcast_to([U, DT, N])
        C_view = bc_sb[:, N:2 * N].unsqueeze(1).broadcast_to([U, DT, N])
        for i in range(NT):
            sl = slice(i * DT, (i + 1) * DT)
            abc = apool.tile([U, DT, N], fp32, tag="abc")
            a_src = a_d[sl, :].unsqueeze(0).broadcast_to([U, DT, N])
            nc.sync.dma_start(abc[:], a_src)
            ht = hpool.tile([U, DT, N], fp32, tag="ht")
            nc.sync.dma_start(ht[:], h_d[:, sl, :])
            # tmp = delta (x) A   (in place over abc)
            dview = delta_sb[:, sl].unsqueeze(2).broadcast_to([U, DT, N])
            nc.vector.tensor_tensor(abc[:], dview, abc[:], op=OP.mult)
            # abar = exp(tmp)  (in place)
            nc.scalar.activation(abc[:], abc[:], AF.Exp)
            # ah = abar * h    (in place over ht)
            nc.vector.tensor_tensor(ht[:], abc[:], ht[:], op=OP.mult)
            # bx = dx (x) B
            dxview = dx_sb[:, sl].unsqueeze(2).broadcast_to([U, DT, N])
            bxt = bpool.tile([U, DT, N], fp32, tag="bxt")
            nc.vector.tensor_tensor(bxt[:], dxview, B_view, op=OP.mult)
            # h_new = ah + bx
            nc.vector.tensor_tensor(ht[:], bxt[:], ht[:], op=OP.add)
            # q = h_new * C  (into abc)
            nc.vector.tensor_tensor(abc[:], ht[:], C_view, op=OP.mult)
            # y[:, sl] += sum_n q
            yp = ypool.tile([U, DT], fp32, tag="yp")
            nc.vector.tensor_reduce(yp[:], abc[:], axis=mybir.AxisListType.X, op=OP.add)
            nc.vector.tensor_tensor(y_sb[:, sl], yp[:], y_sb[:, sl], op=OP.add)
            nc.sync.dma_start(hnew_d[:, sl, :], ht[:])
        nc.sync.dma_start(y_d, y_sb[:])

    nc.compile()
    return nc


def _get_modules():
    if "proj" not in _cache:
        _cache["proj"] = _build_proj()
        _cache["main"] = _build_main()
    return _cache["proj"], _cache["main"]


def _run(nc, in_maps, trace=False):
    from concourse import bass_utils
    return bass_utils.run_bass_kernel_spmd(
        nc, in_maps, core_ids=list(range(NCORES)), trace=trace,
    )


def _prep(x, h, W_delta, W_dt, b_dt, W_B, W_C, A, D):
    x = np.ascontiguousarray(np.asarray(x, np.float32))
    h = np.asarray(h, np.float32)
    wall = np.ascontiguousarray(
        np.concatenate(
            [np.asarray(W_delta, np.float32), np.asarray(W_B, np.float32),
             np.asarray(W_C, np.float32)], axis=1)
    )
    xt = np.ascontiguousarray(x.T)
    wdt_aug = np.ascontiguousarray(
        np.concatenate(
            [np.asarray(W_dt, np.float32),
             np.asarray(b_dt, np.float32)[None, :]], axis=0)
    )
    A = np.asarray(A, np.float32)
    D = np.asarray(D, np.float32)
    proj_maps = []
    for k in range(NCORES):
        proj_maps.append({
            "xT": xt,
            "wsl": np.ascontiguousarray(wall[:, k * WSL:(k + 1) * WSL]),
        })
    main_common = {"x": x, "h": h, "wdt_aug": wdt_aug, "A": A, "D": D}
    return proj_maps, main_common


def _main_maps(common, tbc):
    x, h, wdt_aug, A, D = (common["x"], common["h"], common["wdt_aug"],
                           common["A"], common["D"])
    t = tbc[:, 0:RANK]
    bc = np.ascontiguousarray(tbc[:, RANK:W2])
    tta = np.ascontiguousarray(
        np.concatenate([t.T, np.ones((1, U), np.float32)], axis=0))
    in_maps = []
    for k in range(NCORES):
        sl = slice(k * DSH, (k + 1) * DSH)
        in_maps.append({
            "h_in": np.ascontiguousarray(h[:, sl, :]),
            "x_sh": np.ascontiguousarray(x[:, sl]),
            "tTa": tta,
            "bc_in": bc,
            "wdt_aug": np.ascontiguousarray(wdt_aug[:, sl]),
            "a_sh": np.ascontiguousarray(A[sl, :]),
            "dbc": np.ascontiguousarray(
                np.broadcast_to(D[sl][None, :], (U, DSH))),
        })
    return in_maps


def _gather(results):
    y = np.concatenate([results[k]["y_out"] for k in range(NCORES)], axis=1)
    h_new = np.concatenate([results[k]["h_out"] for k in range(NCORES)], axis=1)
    return y.astype(np.float32), h_new.astype(np.float32)


def _run_all(inputs, trace=False):
    nc_proj, nc_main = _get_modules()
    proj_maps, common = _prep(**inputs)
    res1 = _run(nc_proj, proj_maps, trace=trace)
    tbc = np.concatenate(
        [res1.results[k]["tbc_out"] for k in range(NCORES)], axis=1)
    res2 = _run(nc_main, _main_maps(common, tbc), trace=trace)
    return _gather(res2.results), res1, res2


def kernel(x, h, W_delta, W_dt, b_dt, W_B, W_C, A, D):
    (y, h_new), _, _ = _run_all(dict(
        x=x, h=h, W_delta=W_delta, W_dt=W_dt, b_dt=b_dt,
        W_B=W_B, W_C=W_C, A=A, D=D), trace=False)
    return y, h_new


def _install_ntff_hook():
    """Shim antenv.axon_hooks (absent in this image) and register the
    ctypes NTFF profile hook so trace=True yields exec_time_ns."""
    import sys
    import types
    if "antenv.axon_hooks" not in sys.modules:
        import antenv
        mod = types.ModuleType("antenv.axon_hooks")
        mod._hook = None

        def set_axon_ntff_profile_hook(h):
            mod._hook = h

        def get_axon_ntff_profile_hook():
            return mod._hook

        mod.set_axon_ntff_profile_hook = set_axon_ntff_profile_hook
        mod.get_axon_ntff_profile_hook = get_axon_ntff_profile_hook
        sys.modules["antenv.axon_hooks"] = mod
        antenv.axon_hooks = mod
    import antenv.axon_hooks as ah
    if ah._hook is None:
        from trn_agent_boot.trn_boot import _ntff_profile_via_ctypes
        hook = _ntff_profile_via_ctypes("/opt/axon/libaxon_pjrt.so")
        if hook is not None:
            ah.set_axon_ntff_profile_hook(hook)
    from concourse import bass_utils
    bass_utils.upload_artifacts = lambda tmpdir: f"local:{tmpdir}"


def kernel_traced(x, h, W_delta, W_dt, b_dt, W_B, W_C, A, D):
    """Like kernel() but with NTFF tracing; returns ((y, h_new), res1, res2)."""
    _install_ntff_hook()
    out, res1, res2 = _run_all(dict(
        x=x, h=h, W_delta=W_delta, W_dt=W_dt, b_dt=b_dt,
        W_B=W_B, W_C=W_C, A=A, D=D), trace=True)
    return out, res1, res2


# revision 25
# speedup vs baseline: 1.4362x; 1.1286x over previous
"""Trainium2 Bass kernel for a Mamba-style SSM single step.

Reference math (fp32):
    delta = softplus(x @ W_delta @ W_dt + b_dt)        [U, D]
    B = x @ W_B ; C = x @ W_C                          [U, N]
    abar = exp(delta[:,:,None] * A[None,:,:])          [U, D, N]
    h_new = abar * h + (delta*x)[:,:,None] * B[:,None,:]
    y = einsum('udn,un->ud', h_new, C) + D_vec * x

Two SPMD launches over 8 cores:
  1. proj: [t|B|C] = x @ [W_delta|W_B|W_C], column-sharded (48/core).
     Host gathers the 128x384 result and transposes t.
  2. main: tensor-parallel over d_inner (1280/core): delta = softplus
     (t @ W_dt + b), then the elementwise state update in a layout with
     partition dim = users (U=128), free = (d, n).
"""

import os
import numpy as np

U, D_IN, RANK, N = 128, 10240, 320, 32
NCORES = 8
DSH = D_IN // NCORES            # 1280 per-core d shard
DT = int(os.environ.get("MAMBA_DT", "128"))   # d-tile size
NT = DSH // DT                  # tiles per core
CH = D_IN // 128                # contraction chunks for projections
W2 = RANK + 2 * N               # 384
WSL = W2 // NCORES              # 48 proj columns per core

_cache = {}
CHS = DSH // 128                # 10 contraction chunks per core in proj


def _register_scan_op():
    """Register a custom DVE op: out = cumsum(in0 * in1) along the free
    stream (per partition). uops sha is computed at registration."""
    from concourse import dve_ops
    from concourse.dve_spec import Spec, Src0, Src1, scan, AluOp, lower, _has_src1
    from concourse.dve_uop import DveOpSpec

    if hasattr(dve_ops, "MAMBA_MSUM"):
        return dve_ops.MAMBA_MSUM

    def _ref(in0, in1, s0, s1, imm2):
        P = in0.shape[0]
        a = np.asarray(in0, np.float32).reshape(P, -1)
        b = np.asarray(in1, np.float32).reshape(P, -1)
        return np.cumsum(a * b, axis=1, dtype=np.float32)

    spec = Spec(body=scan(AluOp.ADD, Src0 * Src1), reference=_ref)
    op = dve_ops.DveOp("MAMBA_MSUM", spec, subdim=False, uops_sha={})
    dve_ops.OPS.append(op)
    dve_ops.CUSTOM_DVE_SPECS[op.name] = spec
    dve_ops._SUB_OPCODE_FOR_NAME[op.name] = (
        dve_ops._CUSTOM_DVE_ROW_BASE + len(dve_ops.OPS) - 1)
    for ver in ("v3", "v4"):
        ds = DveOpSpec(
            name=op.name,
            opcode=dve_ops.get_dve_sub_opcode(op.name),
            uops=lower(spec, ver=ver),
            rd1_en=_has_src1(spec),
        )
        op.uops_sha[ver] = ds.sha(ver)
    dve_ops.MAMBA_MSUM = op
    return op


def _build_proj():
    import concourse.mybir as mybir
    import concourse.tile as tile
    from concourse import bacc
    from contextlib import ExitStack

    fp32 = mybir.dt.float32
    nc = bacc.Bacc("TRN2", target_bir_lowering=False, debug=False,
                   enable_asserts=False, num_devices=NCORES)
    # contraction-split: each core contracts its own 1280-row slice of
    # x^T and W_all over ALL 384 output columns; host sums the partials.
    xt_d = nc.dram_tensor("xTs", [DSH, U], fp32, kind="ExternalInput").ap()
    w_d = nc.dram_tensor("wsl", [DSH, W2], fp32, kind="ExternalInput").ap()
    o_d = nc.dram_tensor("tbc_out", [128, W2], fp32, kind="ExternalOutput").ap()

    with tile.TileContext(nc) as tc, ExitStack() as ctx:
        wpool = ctx.enter_context(tc.tile_pool(name="w", bufs=1))
        ppool = ctx.enter_context(tc.tile_pool(name="ps", bufs=1, space="PSUM"))
        spool = ctx.enter_context(tc.tile_pool(name="s", bufs=1))
        ps = ppool.tile([128, W2], fp32, tag="acc")
        xt_c = wpool.tile([128, CHS, U], fp32, tag="xt")
        nc.sync.dma_start(xt_c[:], xt_d.rearrange("(k p) u -> p k u", p=128))
        w_c = wpool.tile([128, CHS, W2], fp32, tag="w")
        nc.sync.dma_start(w_c[:], w_d.rearrange("(k p) w -> p k w", p=128))
        for k in range(CHS):
            nc.tensor.matmul(ps[:], lhsT=xt_c[:, k, :], rhs=w_c[:, k, :],
                             start=(k == 0), stop=(k == CHS - 1))
        out_sb = spool.tile([128, W2], fp32, tag="out")
        nc.scalar.copy(out_sb[:], ps[:])
        nc.sync.dma_start(o_d, out_sb[:])
    nc.compile()
    return nc


def _build_main():
    import concourse.mybir as mybir
    import concourse.tile as tile
    from concourse import bacc
    from contextlib import ExitStack

    fp32 = mybir.dt.float32
    AF = mybir.ActivationFunctionType
    OP = mybir.AluOpType
    scan_op = _register_scan_op()

    nc = bacc.Bacc("TRN2", target_bir_lowering=False, debug=False,
                   enable_asserts=False, num_devices=NCORES)

    h_d = nc.dram_tensor("h_in", [U, DSH, N], fp32, kind="ExternalInput").ap()
    x_d = nc.dram_tensor("x_sh", [U, DSH], fp32, kind="ExternalInput").ap()
    tta_d = nc.dram_tensor("tTa", [RANK + 1, U], fp32, kind="ExternalInput").ap()
    bc_d = nc.dram_tensor("bc_in", [128, 2 * N], fp32, kind="ExternalInput").ap()
    wdt_d = nc.dram_tensor("wdt_aug", [RANK + 1, DSH], fp32, kind="ExternalInput").ap()
    a_d = nc.dram_tensor("a_sh", [DSH, N], fp32, kind="ExternalInput").ap()
    dbc_d = nc.dram_tensor("dbc", [U, DSH], fp32, kind="ExternalInput").ap()
    hnew_d = nc.dram_tensor("h_out", [U, DSH, N], fp32, kind="ExternalOutput").ap()
    y_d = nc.dram_tensor("y_out", [U, DSH], fp32, kind="ExternalOutput").ap()

    with tile.TileContext(nc) as tc, ExitStack() as ctx:
        const = ctx.enter_context(tc.tile_pool(name="const", bufs=1))
        ppool = ctx.enter_context(tc.tile_pool(name="ps", bufs=2, space="PSUM"))
        hpool = ctx.enter_context(tc.tile_pool(name="h", bufs=2))
        apool = ctx.enter_context(tc.tile_pool(name="abc", bufs=2))
        bpool = ctx.enter_context(tc.tile_pool(name="bx", bufs=2))

        # ---------------- Phase P: delta projection ----------------
        x_sb = const.tile([U, DSH], fp32, tag="x")
        nc.sync.dma_start(x_sb[:], x_d)
        dbc_sb = const.tile([U, DSH], fp32, tag="dbc")
        nc.sync.dma_start(dbc_sb[:], dbc_d)
        wdt0 = const.tile([128, DSH], fp32, tag="wdt0")
        nc.sync.dma_start(wdt0[:], wdt_d[0:128, :])
        wdt1 = const.tile([128, DSH], fp32, tag="wdt1")
        nc.sync.dma_start(wdt1[:], wdt_d[128:256, :])
        wdt2 = const.tile([65, DSH], fp32, tag="wdt2")
        nc.sync.dma_start(wdt2[:], wdt_d[256:321, :])
        tT0 = const.tile([128, U], fp32, tag="tT0")
        nc.sync.dma_start(tT0[:], tta_d[0:128, :])
        tT1 = const.tile([128, U], fp32, tag="tT1")
        nc.sync.dma_start(tT1[:], tta_d[128:256, :])
        tT2 = const.tile([65, U], fp32, tag="tT2")
        nc.sync.dma_start(tT2[:], tta_d[256:321, :])
        bc_sb = const.tile([128, 2 * N], fp32, tag="bc")
        nc.sync.dma_start(bc_sb[:], bc_d)

        # delta = softplus(tT.T @ W_dt_aug): per-tile Exp, one Ln pass
        delta_sb = const.tile([U, DSH], fp32, tag="delta")
        for i in range(NT):
            sl = slice(i * DT, (i + 1) * DT)
            d_ps = ppool.tile([U, DT], fp32, tag="dps")
            nc.tensor.matmul(d_ps[:], lhsT=tT0[:], rhs=wdt0[:, sl], start=True, stop=False)
            nc.tensor.matmul(d_ps[:], lhsT=tT1[:], rhs=wdt1[:, sl], start=False, stop=False)
            nc.tensor.matmul(d_ps[:], lhsT=tT2[:], rhs=wdt2[:, sl], start=False, stop=True)
            if os.environ.get("MAMBA_ONELN", "1") == "1":
                nc.scalar.activation(delta_sb[:, sl], d_ps[:], AF.Exp)
            else:
                nc.scalar.activation(d_ps[:], d_ps[:], AF.Exp)
                nc.scalar.activation(delta_sb[:, sl], d_ps[:], AF.Ln, bias=1.0)
        if os.environ.get("MAMBA_ONELN", "1") == "1":
            nc.scalar.activation(delta_sb[:], delta_sb[:], AF.Ln, bias=1.0)

        # dx = delta * x ; y init = D * x
        dx_sb = const.tile([U, DSH], fp32, tag="dx")
        nc.vector.tensor_tensor(dx_sb[:], delta_sb[:], x_sb[:], op=OP.mult)
        y_sb = const.tile([U, DSH], fp32, tag="y")
        nc.vector.tensor_tensor(y_sb[:], x_sb[:], dbc_sb[:], op=OP.mult)

        # ---------------- Phase E: state update ----------------
        B_view = bc_sb[:, 0:N].unsqueeze(1).broadcast_to([U, DT, N])
        C_view = bc_sb[:, N:2 * N].unsqueeze(1).broadcast_to([U, DT, N])
        for i in range(NT):
            sl = slice(i * DT, (i + 1) * DT)
            abc = apool.tile([U, DT, N], fp32, tag="abc")
            a_src = a_d[sl, :].unsqueeze(0).broadcast_to([U, DT, N])
            nc.sync.dma_start(abc[:], a_src)
            ht = hpool.tile([U, DT, N], fp32, tag="ht")
            nc.sync.dma_start(ht[:], h_d[:, sl, :])
            # tmp = delta (x) A   (in place over abc)
            dview = delta_sb[:, sl].unsqueeze(2).broadcast_to([U, DT, N])
            nc.vector.tensor_tensor(abc[:], dview, abc[:], op=OP.mult)
            # abar = exp(tmp)  (in place)
            nc.scalar.activation(abc[:], abc[:], AF.Exp)
            # ah = abar * h    (in place over ht)
            nc.vector.tensor_tensor(ht[:], abc[:], ht[:], op=OP.mult)
            # bx = dx (x) B
            dxview = dx_sb[:, sl].unsqueeze(2).broadcast_to([U, DT, N])
            bxt = bpool.tile([U, DT, N], fp32, tag="bxt")
            nc.vector.tensor_tensor(bxt[:], dxview, B_view, op=OP.mult)
            # h_new = ah + bx
            nc.vector.tensor_tensor(ht[:], bxt[:], ht[:], op=OP.add)
            if os.environ.get("MAMBA_SCAN", "1") == "1":
                # prefix = cumsum(h_new * C) along the tile's free stream
                # (into abc); per-d sums are prefix[d,N-1] - prefix[d-1,N-1].
                nc.vector._custom_dve(scan_op, out=abc[:], in0=ht[:], in1=C_view)
                s_last = abc[:, :, N - 1]
                nc.vector.tensor_tensor(y_sb[:, sl], s_last, y_sb[:, sl], op=OP.add)
                nc.vector.tensor_tensor(
                    y_sb[:, i * DT + 1:(i + 1) * DT],
                    y_sb[:, i * DT + 1:(i + 1) * DT],
                    abc[:, 0:DT - 1, N - 1],
                    op=OP.subtract,
                )
            else:
                nc.vector.tensor_tensor(abc[:], ht[:], C_view, op=OP.mult)
                yp = bpool.tile([U, DT], fp32, tag="yp")
                nc.vector.tensor_reduce(yp[:], abc[:], axis=mybir.AxisListType.X, op=OP.add)
                nc.vector.tensor_tensor(y_sb[:, sl], yp[:], y_sb[:, sl], op=OP.add)
            nc.sync.dma_start(hnew_d[:, sl, :], ht[:])
        nc.sync.dma_start(y_d, y_sb[:])

    nc.compile()
    return nc


def _get_modules():
    if "proj" not in _cache:
        _cache["proj"] = _build_proj()
        _cache["main"] = _build_main()
    return _cache["proj"], _cache["main"]


def _run(nc, in_maps, trace=False):
    from concourse import bass_utils
    return bass_utils.run_bass_kernel_spmd(
        nc, in_maps, core_ids=list(range(NCORES)), trace=trace,
    )


def _prep(x, h, W_delta, W_dt, b_dt, W_B, W_C, A, D):
    x = np.ascontiguousarray(np.asarray(x, np.float32))
    h = np.asarray(h, np.float32)
    wall = np.ascontiguousarray(
        np.concatenate(
            [np.asarray(W_delta, np.float32), np.asarray(W_B, np.float32),
             np.asarray(W_C, np.float32)], axis=1)
    )
    xt = np.ascontiguousarray(x.T)
    wdt_aug = np.ascontiguousarray(
        np.concatenate(
            [np.asarray(W_dt, np.float32),
             np.asarray(b_dt, np.float32)[None, :]], axis=0)
    )
    A = np.asarray(A, np.float32)
    D = np.asarray(D, np.float32)
    proj_maps = []
    for k in range(NCORES):
        sl = slice(k * DSH, (k + 1) * DSH)
        proj_maps.append({
            "xTs": np.ascontiguousarray(xt[sl, :]),
            "wsl": np.ascontiguousarray(wall[sl, :]),
        })
    main_common = {"x": x, "h": h, "wdt_aug": wdt_aug, "A": A, "D": D}
    return proj_maps, main_common


def _main_maps(common, tbc):
    x, h, wdt_aug, A, D = (common["x"], common["h"], common["wdt_aug"],
                           common["A"], common["D"])
    t = tbc[:, 0:RANK]
    bc = np.ascontiguousarray(tbc[:, RANK:W2])
    tta = np.ascontiguousarray(
        np.concatenate([t.T, np.ones((1, U), np.float32)], axis=0))
    in_maps = []
    for k in range(NCORES):
        sl = slice(k * DSH, (k + 1) * DSH)
        in_maps.append({
            "h_in": np.ascontiguousarray(h[:, sl, :]),
            "x_sh": np.ascontiguousarray(x[:, sl]),
            "tTa": tta,
            "bc_in": bc,
            "wdt_aug": np.ascontiguousarray(wdt_aug[:, sl]),
            "a_sh": np.ascontiguousarray(A[sl, :]),
            "dbc": np.ascontiguousarray(
                np.broadcast_to(D[sl][None, :], (U, DSH))),
        })
    return in_maps


def _gather(results):
    y = np.concatenate([results[k]["y_out"] for k in range(NCORES)], axis=1)
    h_new = np.concatenate([results[k]["h_out"] for k in range(NCORES)], axis=1)
    return y.astype(np.float32), h_new.astype(np.float32)


def _run_all(inputs, trace=False):
    nc_proj, nc_main = _get_modules()
    proj_maps, common = _prep(**inputs)
    res1 = _run(nc_proj, proj_maps, trace=trace)
    tbc = np.sum([res1.results[k]["tbc_out"] for k in range(NCORES)],
                 axis=0, dtype=np.float64).astype(np.float32)
    res2 = _run(nc_main, _main_maps(common, tbc), trace=trace)
    return _gather(res2.results), res1, res2


def kernel(x, h, W_delta, W_dt, b_dt, W_B, W_C, A, D):
    (y, h_new), _, _ = _run_all(dict(
        x=x, h=h, W_delta=W_delta, W_dt=W_dt, b_dt=b_dt,
        W_B=W_B, W_C=W_C, A=A, D=D), trace=False)
    return y, h_new


def _install_ntff_hook():
    """Shim antenv.axon_hooks (absent in this image) and register the
    ctypes NTFF profile hook so trace=True yields exec_time_ns."""
    import sys
    import types
    if "antenv.axon_hooks" not in sys.modules:
        import antenv
        mod = types.ModuleType("antenv.axon_hooks")
        mod._hook = None

        def set_axon_ntff_profile_hook(h):
            mod._hook = h

        def get_axon_ntff_profile_hook():
            return mod._hook

        mod.set_axon_ntff_profile_hook = set_axon_ntff_profile_hook
        mod.get_axon_ntff_profile_hook = get_axon_ntff_profile_hook
        sys.modules["antenv.axon_hooks"] = mod
        antenv.axon_hooks = mod
    import antenv.axon_hooks as ah
    if ah._hook is None:
        from trn_agent_boot.trn_boot import _ntff_profile_via_ctypes
        hook = _ntff_profile_via_ctypes("/opt/axon/libaxon_pjrt.so")
        if hook is not None:
            ah.set_axon_ntff_profile_hook(hook)
    from concourse import bass_utils
    bass_utils.upload_artifacts = lambda tmpdir: f"local:{tmpdir}"


def kernel_traced(x, h, W_delta, W_dt, b_dt, W_B, W_C, A, D):
    """Like kernel() but with NTFF tracing; returns ((y, h_new), res1, res2)."""
    _install_ntff_hook()
    out, res1, res2 = _run_all(dict(
        x=x, h=h, W_delta=W_delta, W_dt=W_dt, b_dt=b_dt,
        W_B=W_B, W_C=W_C, A=A, D=D), trace=True)
    return out, res1, res2


# revision 26
# speedup vs baseline: 1.5388x; 1.0714x over previous
"""Trainium2 Bass kernel for a Mamba-style SSM single step.

Reference math (fp32):
    delta = softplus(x @ W_delta @ W_dt + b_dt)        [U, D]
    B = x @ W_B ; C = x @ W_C                          [U, N]
    abar = exp(delta[:,:,None] * A[None,:,:])          [U, D, N]
    h_new = abar * h + (delta*x)[:,:,None] * B[:,None,:]
    y = einsum('udn,un->ud', h_new, C) + D_vec * x

Two SPMD launches over 8 cores:
  1. proj: [t|B|C] = x @ [W_delta|W_B|W_C], column-sharded (48/core).
     Host gathers the 128x384 result and transposes t.
  2. main: tensor-parallel over d_inner (1280/core): delta = softplus
     (t @ W_dt + b), then the elementwise state update in a layout with
     partition dim = users (U=128), free = (d, n).
"""

import os
import numpy as np

U, D_IN, RANK, N = 128, 10240, 320, 32
NCORES = 8
DSH = D_IN // NCORES            # 1280 per-core d shard
DT = int(os.environ.get("MAMBA_DT", "128"))   # d-tile size
NT = DSH // DT                  # tiles per core
CH = D_IN // 128                # contraction chunks for projections
W2 = RANK + 2 * N               # 384
WSL = W2 // NCORES              # 48 proj columns per core

_cache = {}
CHS = DSH // 128                # 10 contraction chunks per core in proj


def _register_scan_op():
    """Register a custom DVE op: out = cumsum(in0 * in1) along the free
    stream (per partition). uops sha is computed at registration."""
    from concourse import dve_ops
    from concourse.dve_spec import Spec, Src0, Src1, scan, AluOp, lower, _has_src1
    from concourse.dve_uop import DveOpSpec

    if hasattr(dve_ops, "MAMBA_MSUM"):
        return dve_ops.MAMBA_MSUM

    def _ref(in0, in1, s0, s1, imm2):
        P = in0.shape[0]
        a = np.asarray(in0, np.float32).reshape(P, -1)
        b = np.asarray(in1, np.float32).reshape(P, -1)
        return np.cumsum(a * b, axis=1, dtype=np.float32)

    spec = Spec(body=scan(AluOp.ADD, Src0 * Src1), reference=_ref)
    op = dve_ops.DveOp("MAMBA_MSUM", spec, subdim=False, uops_sha={})
    dve_ops.OPS.append(op)
    dve_ops.CUSTOM_DVE_SPECS[op.name] = spec
    dve_ops._SUB_OPCODE_FOR_NAME[op.name] = (
        dve_ops._CUSTOM_DVE_ROW_BASE + len(dve_ops.OPS) - 1)
    for ver in ("v3", "v4"):
        ds = DveOpSpec(
            name=op.name,
            opcode=dve_ops.get_dve_sub_opcode(op.name),
            uops=lower(spec, ver=ver),
            rd1_en=_has_src1(spec),
        )
        op.uops_sha[ver] = ds.sha(ver)
    dve_ops.MAMBA_MSUM = op
    return op


def _build_proj():
    import concourse.mybir as mybir
    import concourse.tile as tile
    from concourse import bacc
    from contextlib import ExitStack

    fp32 = mybir.dt.float32
    nc = bacc.Bacc("TRN2", target_bir_lowering=False, debug=False,
                   enable_asserts=False, num_devices=NCORES)
    # contraction-split: each core contracts its own 1280-row slice of
    # x^T and W_all over ALL 384 output columns; host sums the partials.
    xt_d = nc.dram_tensor("xTs", [DSH, U], fp32, kind="ExternalInput").ap()
    w_d = nc.dram_tensor("wsl", [DSH, W2], fp32, kind="ExternalInput").ap()
    o_d = nc.dram_tensor("tbc_out", [128, W2], fp32, kind="ExternalOutput").ap()

    with tile.TileContext(nc) as tc, ExitStack() as ctx:
        wpool = ctx.enter_context(tc.tile_pool(name="w", bufs=1))
        ppool = ctx.enter_context(tc.tile_pool(name="ps", bufs=1, space="PSUM"))
        spool = ctx.enter_context(tc.tile_pool(name="s", bufs=1))
        ps = ppool.tile([128, W2], fp32, tag="acc")
        xt_c = wpool.tile([128, CHS, U], fp32, tag="xt")
        nc.sync.dma_start(xt_c[:], xt_d.rearrange("(k p) u -> p k u", p=128))
        w_c = wpool.tile([128, CHS, W2], fp32, tag="w")
        nc.sync.dma_start(w_c[:], w_d.rearrange("(k p) w -> p k w", p=128))
        for k in range(CHS):
            nc.tensor.matmul(ps[:], lhsT=xt_c[:, k, :], rhs=w_c[:, k, :],
                             start=(k == 0), stop=(k == CHS - 1))
        out_sb = spool.tile([128, W2], fp32, tag="out")
        nc.scalar.copy(out_sb[:], ps[:])
        nc.sync.dma_start(o_d, out_sb[:])
    nc.compile()
    return nc


def _build_main():
    import concourse.mybir as mybir
    import concourse.tile as tile
    from concourse import bacc
    from contextlib import ExitStack

    fp32 = mybir.dt.float32
    AF = mybir.ActivationFunctionType
    OP = mybir.AluOpType
    scan_op = _register_scan_op()

    nc = bacc.Bacc("TRN2", target_bir_lowering=False, debug=False,
                   enable_asserts=False, num_devices=NCORES)

    h_d = nc.dram_tensor("h_in", [U, DSH, N], fp32, kind="ExternalInput").ap()
    x_d = nc.dram_tensor("x_sh", [U, DSH], fp32, kind="ExternalInput").ap()
    tta_d = nc.dram_tensor("tTa", [RANK + 1, U], fp32, kind="ExternalInput").ap()
    bc_d = nc.dram_tensor("bc_in", [128, 2 * N], fp32, kind="ExternalInput").ap()
    wdt_d = nc.dram_tensor("wdt_aug", [RANK + 1, DSH], fp32, kind="ExternalInput").ap()
    a_d = nc.dram_tensor("a_sh", [DSH, N], fp32, kind="ExternalInput").ap()
    dbc_d = nc.dram_tensor("dbc", [U, DSH], fp32, kind="ExternalInput").ap()
    hnew_d = nc.dram_tensor("h_out", [U, DSH, N], fp32, kind="ExternalOutput").ap()
    y_d = nc.dram_tensor("y_out", [U, DSH], fp32, kind="ExternalOutput").ap()

    with tile.TileContext(nc) as tc, ExitStack() as ctx:
        const = ctx.enter_context(tc.tile_pool(name="const", bufs=1))
        ppool = ctx.enter_context(tc.tile_pool(name="ps", bufs=2, space="PSUM"))
        hpool = ctx.enter_context(tc.tile_pool(name="h", bufs=3))
        apool = ctx.enter_context(tc.tile_pool(name="abc", bufs=3))
        bpool = ctx.enter_context(tc.tile_pool(name="bx", bufs=2))

        # ---------------- Phase P: delta projection ----------------
        x_sb = const.tile([U, DSH], fp32, tag="x")
        nc.sync.dma_start(x_sb[:], x_d)
        dbc_sb = const.tile([U, DSH], fp32, tag="dbc")
        nc.sync.dma_start(dbc_sb[:], dbc_d)
        wdt0 = const.tile([128, DSH], fp32, tag="wdt0")
        nc.sync.dma_start(wdt0[:], wdt_d[0:128, :])
        wdt1 = const.tile([128, DSH], fp32, tag="wdt1")
        nc.sync.dma_start(wdt1[:], wdt_d[128:256, :])
        wdt2 = const.tile([65, DSH], fp32, tag="wdt2")
        nc.sync.dma_start(wdt2[:], wdt_d[256:321, :])
        tT0 = const.tile([128, U], fp32, tag="tT0")
        nc.sync.dma_start(tT0[:], tta_d[0:128, :])
        tT1 = const.tile([128, U], fp32, tag="tT1")
        nc.sync.dma_start(tT1[:], tta_d[128:256, :])
        tT2 = const.tile([65, U], fp32, tag="tT2")
        nc.sync.dma_start(tT2[:], tta_d[256:321, :])
        bc_sb = const.tile([128, 2 * N], fp32, tag="bc")
        nc.sync.dma_start(bc_sb[:], bc_d)

        # delta = softplus(tT.T @ W_dt_aug): per-tile Exp, one Ln pass
        delta_sb = const.tile([U, DSH], fp32, tag="delta")
        for i in range(NT):
            sl = slice(i * DT, (i + 1) * DT)
            d_ps = ppool.tile([U, DT], fp32, tag="dps")
            nc.tensor.matmul(d_ps[:], lhsT=tT0[:], rhs=wdt0[:, sl], start=True, stop=False)
            nc.tensor.matmul(d_ps[:], lhsT=tT1[:], rhs=wdt1[:, sl], start=False, stop=False)
            nc.tensor.matmul(d_ps[:], lhsT=tT2[:], rhs=wdt2[:, sl], start=False, stop=True)
            if os.environ.get("MAMBA_ONELN", "1") == "1":
                nc.scalar.activation(delta_sb[:, sl], d_ps[:], AF.Exp)
            else:
                nc.scalar.activation(d_ps[:], d_ps[:], AF.Exp)
                nc.scalar.activation(delta_sb[:, sl], d_ps[:], AF.Ln, bias=1.0)
        if os.environ.get("MAMBA_ONELN", "1") == "1":
            nc.scalar.activation(delta_sb[:], delta_sb[:], AF.Ln, bias=1.0)

        # dx = delta * x ; y init = D * x
        dx_sb = const.tile([U, DSH], fp32, tag="dx")
        nc.vector.tensor_tensor(dx_sb[:], delta_sb[:], x_sb[:], op=OP.mult)
        y_sb = const.tile([U, DSH], fp32, tag="y")
        nc.vector.tensor_tensor(y_sb[:], x_sb[:], dbc_sb[:], op=OP.mult)

        # ---------------- Phase E: state update ----------------
        B_view = bc_sb[:, 0:N].unsqueeze(1).broadcast_to([U, DT, N])
        C_view = bc_sb[:, N:2 * N].unsqueeze(1).broadcast_to([U, DT, N])
        for i in range(NT):
            sl = slice(i * DT, (i + 1) * DT)
            abc = apool.tile([U, DT, N], fp32, tag="abc")
            a_src = a_d[sl, :].unsqueeze(0).broadcast_to([U, DT, N])
            nc.sync.dma_start(abc[:], a_src)
            ht = hpool.tile([U, DT, N], fp32, tag="ht")
            nc.sync.dma_start(ht[:], h_d[:, sl, :])
            # tmp = delta (x) A   (in place over abc)
            dview = delta_sb[:, sl].unsqueeze(2).broadcast_to([U, DT, N])
            nc.vector.tensor_tensor(abc[:], dview, abc[:], op=OP.mult)
            # abar = exp(tmp)  (in place)
            nc.scalar.activation(abc[:], abc[:], AF.Exp)
            # ah = abar * h    (in place over ht)
            nc.vector.tensor_tensor(ht[:], abc[:], ht[:], op=OP.mult)
            # bx = dx (x) B
            dxview = dx_sb[:, sl].unsqueeze(2).broadcast_to([U, DT, N])
            bxt = bpool.tile([U, DT, N], fp32, tag="bxt")
            nc.vector.tensor_tensor(bxt[:], dxview, B_view, op=OP.mult)
            # h_new = ah + bx
            nc.vector.tensor_tensor(ht[:], bxt[:], ht[:], op=OP.add)
            if os.environ.get("MAMBA_SCAN", "1") == "1":
                # prefix = cumsum(h_new * C) along the tile's free stream
                # (into abc); per-d sums are prefix[d,N-1] - prefix[d-1,N-1].
                nc.vector._custom_dve(scan_op, out=abc[:], in0=ht[:], in1=C_view)
                s_last = abc[:, :, N - 1]
                nc.vector.tensor_tensor(y_sb[:, sl], s_last, y_sb[:, sl], op=OP.add)
                nc.vector.tensor_tensor(
                    y_sb[:, i * DT + 1:(i + 1) * DT],
                    y_sb[:, i * DT + 1:(i + 1) * DT],
                    abc[:, 0:DT - 1, N - 1],
                    op=OP.subtract,
                )
            else:
                nc.vector.tensor_tensor(abc[:], ht[:], C_view, op=OP.mult)
                yp = bpool.tile([U, DT], fp32, tag="yp")
                nc.vector.tensor_reduce(yp[:], abc[:], axis=mybir.AxisListType.X, op=OP.add)
                nc.vector.tensor_tensor(y_sb[:, sl], yp[:], y_sb[:, sl], op=OP.add)
            nc.sync.dma_start(hnew_d[:, sl, :], ht[:])
        nc.sync.dma_start(y_d, y_sb[:])

    nc.compile()
    return nc


def _get_modules():
    if "proj" not in _cache:
        _cache["proj"] = _build_proj()
        _cache["main"] = _build_main()
    return _cache["proj"], _cache["main"]


def _run(nc, in_maps, trace=False):
    from concourse import bass_utils
    return bass_utils.run_bass_kernel_spmd(
        nc, in_maps, core_ids=list(range(NCORES)), trace=trace,
    )


def _prep(x, h, W_delta, W_dt, b_dt, W_B, W_C, A, D):
    x = np.ascontiguousarray(np.asarray(x, np.float32))
    h = np.asarray(h, np.float32)
    wall = np.ascontiguousarray(
        np.concatenate(
            [np.asarray(W_delta, np.float32), np.asarray(W_B, np.float32),
             np.asarray(W_C, np.float32)], axis=1)
    )
    xt = np.ascontiguousarray(x.T)
    wdt_aug = np.ascontiguousarray(
        np.concatenate(
            [np.asarray(W_dt, np.float32),
             np.asarray(b_dt, np.float32)[None, :]], axis=0)
    )
    A = np.asarray(A, np.float32)
    D = np.asarray(D, np.float32)
    proj_maps = []
    for k in range(NCORES):
        sl = slice(k * DSH, (k + 1) * DSH)
        proj_maps.append({
            "xTs": np.ascontiguousarray(xt[sl, :]),
            "wsl": np.ascontiguousarray(wall[sl, :]),
        })
    main_common = {"x": x, "h": h, "wdt_aug": wdt_aug, "A": A, "D": D}
    return proj_maps, main_common


def _main_maps(common, tbc):
    x, h, wdt_aug, A, D = (common["x"], common["h"], common["wdt_aug"],
                           common["A"], common["D"])
    t = tbc[:, 0:RANK]
    bc = np.ascontiguousarray(tbc[:, RANK:W2])
    tta = np.ascontiguousarray(
        np.concatenate([t.T, np.ones((1, U), np.float32)], axis=0))
    in_maps = []
    for k in range(NCORES):
        sl = slice(k * DSH, (k + 1) * DSH)
        in_maps.append({
            "h_in": np.ascontiguousarray(h[:, sl, :]),
            "x_sh": np.ascontiguousarray(x[:, sl]),
            "tTa": tta,
            "bc_in": bc,
            "wdt_aug": np.ascontiguousarray(wdt_aug[:, sl]),
            "a_sh": np.ascontiguousarray(A[sl, :]),
            "dbc": np.ascontiguousarray(
                np.broadcast_to(D[sl][None, :], (U, DSH))),
        })
    return in_maps


def _gather(results):
    y = np.concatenate([results[k]["y_out"] for k in range(NCORES)], axis=1)
    h_new = np.concatenate([results[k]["h_out"] for k in range(NCORES)], axis=1)
    return y.astype(np.float32), h_new.astype(np.float32)


def _run_all(inputs, trace=False):
    nc_proj, nc_main = _get_modules()
    proj_maps, common = _prep(**inputs)
    res1 = _run(nc_proj, proj_maps, trace=trace)
    tbc = np.sum([res1.results[k]["tbc_out"] for k in range(NCORES)],
                 axis=0, dtype=np.float64).astype(np.float32)
    res2 = _run(nc_main, _main_maps(common, tbc), trace=trace)
    return _gather(res2.results), res1, res2


def kernel(x, h, W_delta, W_dt, b_dt, W_B, W_C, A, D):
    (y, h_new), _, _ = _run_all(dict(
        x=x, h=h, W_delta=W_delta, W_dt=W_dt, b_dt=b_dt,
        W_B=W_B, W_C=W_C, A=A, D=D), trace=False)
    return y, h_new


def _install_ntff_hook():
    """Shim antenv.axon_hooks (absent in this image) and register the
    ctypes NTFF profile hook so trace=True yields exec_time_ns."""
    import sys
    import types
    if "antenv.axon_hooks" not in sys.modules:
        import antenv
        mod = types.ModuleType("antenv.axon_hooks")
        mod._hook = None

        def set_axon_ntff_profile_hook(h):
            mod._hook = h

        def get_axon_ntff_profile_hook():
            return mod._hook

        mod.set_axon_ntff_profile_hook = set_axon_ntff_profile_hook
        mod.get_axon_ntff_profile_hook = get_axon_ntff_profile_hook
        sys.modules["antenv.axon_hooks"] = mod
        antenv.axon_hooks = mod
    import antenv.axon_hooks as ah
    if ah._hook is None:
        from trn_agent_boot.trn_boot import _ntff_profile_via_ctypes
        hook = _ntff_profile_via_ctypes("/opt/axon/libaxon_pjrt.so")
        if hook is not None:
            ah.set_axon_ntff_profile_hook(hook)
    from concourse import bass_utils
    bass_utils.upload_artifacts = lambda tmpdir: f"local:{tmpdir}"


def kernel_traced(x, h, W_delta, W_dt, b_dt, W_B, W_C, A, D):
    """Like kernel() but with NTFF tracing; returns ((y, h_new), res1, res2)."""
    _install_ntff_hook()
    out, res1, res2 = _run_all(dict(
        x=x, h=h, W_delta=W_delta, W_dt=W_dt, b_dt=b_dt,
        W_B=W_B, W_C=W_C, A=A, D=D), trace=True)
    return out, res1, res2
